# revision 56
# baseline (speedup 1.0000x reference)
"""Trainium2 Bass kernel for DigitConvolutionalModel forward pass.

Model: x[B,784] -> 3x3 valid conv (28x28 -> 26x26) -> flatten[676]
       -> Linear(676->200) + ReLU -> Linear(200->10).

Key algebraic optimization: the conv is linear and feeds straight into the
first Linear, so both fold into a single effective weight
W_eff[200,784] = w0 compose conv  (computed once on host, ~1.2 MFLOP).
The device then runs two dense GEMMs per batch shard:
    h = relu(x @ W_eff.T + b0);  out = h @ w1.T + b1

Sharding: pure data parallel over the batch dim across 8 NeuronCores
(4096 rows each); weights replicated; no collectives (forward only).

On-device layout is feature-major ("transposed") so the contraction dim
always lives on SBUF partitions: xT[784,n] -> hT[200,n] -> outT[10,n].
The host pre-packs x shards into exact SBUF tile images (k tiled 7x112)
so all x traffic is a handful of large single-ring DMAs whose partition
lines are multi-KB contiguous runs; group sizes are staggered (small
first) so compute starts early. Compute dtype bf16 (1 cyc/row matmuls,
half the DMA bytes); PSUM accumulates f32; bias+ReLU fused on the
vector engine; weights load on the ACT ring; output stores on SWDGE.
Dummy matmuls on zeroed scratch pre-warm the PE's HAM clock gate during
the first DMA's flight.
"""

import os
import sys
import types
import numpy as np

for _p in ("/opt/trn_rl_repo", "/root/.axon_site"):
    if os.path.isdir(_p) and _p not in sys.path:
        sys.path.insert(0, _p)

import concourse.bass as bass  # noqa: E402
import concourse.tile as tile  # noqa: E402
import concourse.mybir as mybir  # noqa: E402
from concourse import bacc  # noqa: E402
from concourse.bass_utils import run_bass_kernel_spmd  # noqa: E402

B = 32768
N_CORES = 8
SHARD = B // N_CORES          # 4096
KDIM = 784                    # 28*28 input features (conv folded in)
HID = 200
OUT = 10
CHUNK = 512                   # batch columns per matmul (moving free dim)
# batch-column widths per pipeline segment: narrow at the head (compute
# starts sooner, bridging the PE warm-up) and at the tail (shorter
# relu->fc2->store latency chain after the last big matmul)
SEGS = [256] + [512] * 7 + [256]
KT = 112                      # k-tile partition size (7 * 112 = 784)
NKT = KDIM // KT              # 7 k-tiles
M_TILES = [(0, 128), (128, 72)]  # hidden 200 = 128 + 72 PSUM partition tiles
N_WARMUP = 13                 # dummy matmuls to trip the HAM clock gate

# matmul operand dtype:
#   float32  — exact, 4 cyc/row           (~143us)
#   float32r — ~2-3 cyc/row, rel err 2e-4 (~71us)
#   bfloat16 — 1 cyc/row, half DMA bytes, rel err ~3e-3
MM_DT = mybir.dt.bfloat16

last_exec_time_ns = None      # set when BASS_KERNEL_PROFILE=1


def _install_ntff_hook():
    """Register the axon NTFF profile hook if the image's antenv lacks it."""
    try:
        from antenv.axon_hooks import get_axon_ntff_profile_hook  # noqa: F401
        return
    except ImportError:
        pass
    try:
        from trn_agent_boot.trn_boot import _ntff_profile_via_ctypes
        hook = _ntff_profile_via_ctypes("/opt/axon/libaxon_pjrt.so")
    except Exception:
        hook = None
    mod = types.ModuleType("antenv.axon_hooks")
    mod.get_axon_ntff_profile_hook = lambda: hook
    mod.set_axon_ntff_profile_hook = lambda h: None
    sys.modules["antenv.axon_hooks"] = mod


def _np_mm_dtype():
    if MM_DT == mybir.dt.bfloat16:
        import ml_dtypes
        return np.dtype(ml_dtypes.bfloat16)
    return np.dtype(np.float32)


def fold_conv_into_fc(conv_w: np.ndarray, w0: np.ndarray) -> np.ndarray:
    """W_eff[200,784] such that x @ W_eff.T == fc1(flatten(conv(x)))."""
    w0v = w0.reshape(HID, 26, 26).astype(np.float64)
    w_img = np.zeros((HID, 28, 28), dtype=np.float64)
    for ki in range(3):
        for kj in range(3):
            w_img[:, ki:ki + 26, kj:kj + 26] += w0v * np.float64(conv_w[ki, kj])
    return w_img.reshape(HID, KDIM).astype(np.float32)


def pack_shard(xs: np.ndarray, mm_np):
    """Pack one x shard [4096, 784] into per-group SBUF tile images.

    Group g (gsz chunks starting at chunk c0):
      xg[p, j, a, n] = x[(c0+j)*CHUNK + n, a*KT + p]
    Every SBUF partition line is one contiguous (j, a, n) run.
    """
    xsv = xs.reshape(SHARD, NKT, KT)
    arrays = []
    c0 = 0
    for w in SEGS:
        blk = xsv[c0:c0 + w]                        # [n, a, p]
        arrays.append(np.ascontiguousarray(
            blk.transpose(2, 1, 0).astype(mm_np)))  # [p, a, n]
        c0 += w
    return arrays


def pack_weights(w_eff: np.ndarray, w1: np.ndarray, b0, b1, mm_np):
    """Pack weights/biases into single-DMA SBUF images."""
    # w0sb[p, a, m] = W_eff[m, a*KT + p]
    w0sb = np.ascontiguousarray(
        w_eff.reshape(HID, NKT, KT).transpose(2, 1, 0).astype(mm_np))
    # w1sb[p, 0:10] = w1[:, p].T ; w1sb[0:72, 10:20] = w1[:, 128+p].T
    w1sb = np.zeros((128, 2 * OUT), dtype=mm_np)
    w1sb[:, :OUT] = w1[:, 0:128].T.astype(mm_np)
    w1sb[:HID - 128, OUT:] = w1[:, 128:HID].T.astype(mm_np)
    # bias[p, 0] = b0[p]; bias[0:72, 1] = b0[128:200]; bias[0:10, 2] = b1
    biases = np.zeros((128, 3), dtype=np.float32)
    biases[:, 0] = b0[0:128]
    biases[:HID - 128, 1] = b0[128:HID]
    biases[:OUT, 2] = b1
    return w0sb, w1sb, biases


def build_program():
    nc = bacc.Bacc("TRN2", target_bir_lowering=False, debug=False)
    f32 = mybir.dt.float32
    add = mybir.AluOpType.add
    amax = mybir.AluOpType.max

    xg_d = [
        nc.declare_dram_parameter(
            f"xg{g}", [KT, NKT, w], MM_DT, isOutput=False)
        for g, w in enumerate(SEGS)
    ]
    w0_d = nc.declare_dram_parameter("w0sb", [KT, NKT, HID], MM_DT, isOutput=False)
    w1_d = nc.declare_dram_parameter("w1sb", [128, 2 * OUT], MM_DT, isOutput=False)
    bia_d = nc.declare_dram_parameter("biases", [128, 3], f32, isOutput=False)
    out_d = nc.declare_dram_parameter("out", [OUT, SHARD], f32, isOutput=True)

    with tile.TileContext(nc) as tc:
        with (
            tc.tile_pool(name="weights", bufs=1) as wpool,
            tc.tile_pool(name="xin", bufs=3) as xpool,
            tc.tile_pool(name="hbuf", bufs=2) as hpool,
            tc.tile_pool(name="obuf", bufs=4) as opool,
            tc.tile_pool(name="psum", bufs=2, space=bass.MemorySpace.PSUM) as pp,
            tc.tile_pool(name="opsum", bufs=2, space=bass.MemorySpace.PSUM) as op,
        ):
            # weights + biases ride the ACT ring so the SP ring belongs
            # exclusively to the x stream (first-chunk completion time)
            w0 = wpool.tile([KT, NKT, HID], MM_DT)
            nc.scalar.dma_start(w0[:], w0_d[:])
            bia = wpool.tile([128, 3], f32)
            nc.scalar.dma_start(bia[:], bia_d[:])
            w1 = wpool.tile([128, 2 * OUT], MM_DT)
            nc.scalar.dma_start(w1[:], w1_d[:])

            # PE pre-warm on zeroed scratch while the first DMAs fly
            warm_x = wpool.tile([KT, CHUNK], MM_DT)
            nc.gpsimd.memset(warm_x[:], 0.0)
            warm_ps = op.tile([128, CHUNK], f32, tag="warm", bufs=1)
            for _ in range(N_WARMUP):
                nc.tensor.matmul(
                    warm_ps[:], warm_x[:, 0:128], warm_x[:],
                    start=True, stop=True)

            def emit_layer2(g, w, c0, h_tiles):
                # layer 2: outT[10, seg], 2 accumulating matmuls
                o_ps = op.tile([OUT, w], f32, tag="ops", name=f"ops_{g}")
                nc.tensor.matmul(
                    o_ps[:], w1[0:128, 0:OUT], h_tiles[0][:],
                    start=True, stop=False)
                nc.tensor.matmul(
                    o_ps[:], w1[0:HID - 128, OUT:2 * OUT], h_tiles[1][:],
                    start=False, stop=True)
                o_sb = opool.tile([OUT, w], f32, tag="osb", name=f"osb_{g}")
                # bias-add on the scalar engine, off DVE's queue
                nc.scalar.activation(
                    o_sb[:], o_ps[:],
                    mybir.ActivationFunctionType.Identity,
                    bias=bia[0:OUT, 2:3])
                # output store on the SWDGE ring (keeps HWDGE free); the
                # last two ride the idle SP ring — SWDGE's end-of-kernel
                # drain of a just-issued store costs ~2us
                if g >= len(SEGS) - 2:
                    nc.sync.dma_start(out_d[:, c0:c0 + w], o_sb[:])
                else:
                    nc.gpsimd.dma_start(out_d[:, c0:c0 + w], o_sb[:])

            c0 = 0
            x_dmas = []
            pending = None   # layer 2 runs one segment behind layer 1,
            # so the PE never waits on the DVE relu at a seg boundary
            for g, w in enumerate(SEGS):
                xg = xpool.tile([KT, NKT, w], MM_DT, tag="xg",
                                name=f"xg_{g}")
                # mid-stream segments ride the SWDGE ring for extra
                # aggregate bandwidth; the rest use the SP HWDGE ring
                eng = (nc.scalar if g == 1 else
       nc.gpsimd if g in (4, 6) else nc.sync)
                dma = eng.dma_start(xg[:], xg_d[g][:])
                # cap x-DMA queue depth: the SDMA engines round-robin
                # across queued transfers, so deeper queues delay the
                # completion of the segment the PE needs next
                if g >= 2:
                    tile.add_dep_helper(
                        dma.ins, x_dmas[g - 2].ins, sync=True,
                        reason="throttle x DMA in-flight depth to 2")
                x_dmas.append(dma)

                # layer 1: hT[m0:m0+dm, seg], 7 accumulating matmuls
                h_tiles = []
                for mi, (m0, dm) in enumerate(M_TILES):
                    h_ps = pp.tile([dm, w], f32, tag=f"hps{mi}",
                                   name=f"hps_{g}_{mi}")
                    for a in range(NKT):
                        nc.tensor.matmul(
                            h_ps[:],
                            w0[:, a, m0:m0 + dm],
                            xg[:, a, :],
                            start=(a == 0),
                            stop=(a == NKT - 1),
                        )
                    h_sb = hpool.tile([dm, w], MM_DT, tag=f"h{mi}",
                                      name=f"h_{g}_{mi}")
                    # fused bias + relu on the vector engine
                    nc.vector.tensor_scalar(
                        h_sb[:], h_ps[:], bia[0:dm, mi:mi + 1], 0.0,
                        add, amax)
                    h_tiles.append(h_sb)

                if pending is not None:
                    emit_layer2(*pending)
                pending = (g, w, c0, h_tiles)
                c0 += w

            emit_layer2(*pending)

    nc.compile()
    return nc


_program_cache = {}


def _get_program():
    key = (MM_DT, tuple(SEGS), N_WARMUP)
    if key not in _program_cache:
        _program_cache[key] = build_program()
    return _program_cache[key]


def kernel(**inputs: np.ndarray) -> np.ndarray:
    x = np.asarray(inputs["x"], dtype=np.float32)
    conv_w = np.asarray(inputs["conv_w"], dtype=np.float32)
    w0 = np.asarray(inputs["w0"], dtype=np.float32)
    b0 = np.asarray(inputs["b0"], dtype=np.float32)
    w1 = np.asarray(inputs["w1"], dtype=np.float32)
    b1 = np.asarray(inputs["b1"], dtype=np.float32)

    mm_np = _np_mm_dtype()
    w_eff = fold_conv_into_fc(conv_w, w0)
    w0sb, w1sb, biases = pack_weights(w_eff, w1, b0, b1, mm_np)

    in_maps = []
    for i in range(N_CORES):
        xgs = pack_shard(x[i * SHARD:(i + 1) * SHARD], mm_np)
        m = {f"xg{g}": xg for g, xg in enumerate(xgs)}
        m.update({"w0sb": w0sb, "w1sb": w1sb, "biases": biases})
        in_maps.append(m)

    nc = _get_program()

    profile = os.environ.get("BASS_KERNEL_PROFILE", "0") == "1"
    kwargs = {}
    if profile:
        _install_ntff_hook()
        kwargs = dict(trace=True, tmpdir=os.environ.get("BASS_KERNEL_TRACE_DIR"))
    try:
        res = run_bass_kernel_spmd(
            nc, in_maps, core_ids=list(range(N_CORES)), **kwargs)
    except Exception:
        # a previous process can leave a NeuronCore momentarily
        # unrecoverable (NRT_EXEC_UNIT_UNRECOVERABLE); one retry suffices
        import time
        time.sleep(5)
        res = run_bass_kernel_spmd(
            nc, in_maps, core_ids=list(range(N_CORES)), **kwargs)

    global last_exec_time_ns
    last_exec_time_ns = res.exec_time_ns

    out = np.empty((B, OUT), dtype=np.float32)
    for i in range(N_CORES):
        out[i * SHARD:(i + 1) * SHARD] = res.results[i]["out"].T
    return out


# revision 57
# speedup vs baseline: 1.1126x; 1.1126x over previous
"""Trainium2 Bass kernel for DigitConvolutionalModel forward pass.

Model: x[B,784] -> 3x3 valid conv (28x28 -> 26x26) -> flatten[676]
       -> Linear(676->200) + ReLU -> Linear(200->10).

Key algebraic optimization: the conv is linear and feeds straight into the
first Linear, so both fold into a single effective weight
W_eff[200,784] = w0 compose conv  (computed once on host, ~1.2 MFLOP).
The device then runs two dense GEMMs per batch shard:
    h = relu(x @ W_eff.T + b0);  out = h @ w1.T + b1

Sharding: pure data parallel over the batch dim across 8 NeuronCores
(4096 rows each); weights replicated; no collectives (forward only).

On-device layout is feature-major ("transposed") so the contraction dim
always lives on SBUF partitions: xT[784,n] -> hT[200,n] -> outT[10,n].
The host pre-packs x shards into exact SBUF tile images (k tiled 7x112)
so all x traffic is a handful of large single-ring DMAs whose partition
lines are multi-KB contiguous runs; group sizes are staggered (small
first) so compute starts early. Compute dtype bf16 (1 cyc/row matmuls,
half the DMA bytes); PSUM accumulates f32; bias+ReLU fused on the
vector engine; weights load on the ACT ring; output stores on SWDGE.
Dummy matmuls on zeroed scratch pre-warm the PE's HAM clock gate during
the first DMA's flight.
"""

import os
import sys
import types
import numpy as np

for _p in ("/opt/trn_rl_repo", "/root/.axon_site"):
    if os.path.isdir(_p) and _p not in sys.path:
        sys.path.insert(0, _p)

import concourse.bass as bass  # noqa: E402
import concourse.tile as tile  # noqa: E402
import concourse.mybir as mybir  # noqa: E402
from concourse import bacc  # noqa: E402
from concourse.bass_utils import run_bass_kernel_spmd  # noqa: E402

B = 32768
N_CORES = 8
SHARD = B // N_CORES          # 4096
KDIM = 784                    # 28*28 input features (conv folded in)
HID = 200
OUT = 10
CHUNK = 512                   # batch columns per matmul (moving free dim)
# batch-column widths per pipeline segment: narrow at the head (compute
# starts sooner, bridging the PE warm-up) and at the tail (shorter
# relu->fc2->store latency chain after the last big matmul)
SEGS = [256] + [512] * 7 + [256]
KT = 112                      # k-tile partition size (7 * 112 = 784)
NKT = KDIM // KT              # 7 k-tiles
M_TILES = [(0, 128), (128, 72)]  # hidden 200 = 128 + 72 PSUM partition tiles
N_WARMUP = 13                 # dummy matmuls to trip the HAM clock gate

# matmul operand dtype:
#   float32  — exact, 4 cyc/row           (~143us)
#   float32r — ~2-3 cyc/row, rel err 2e-4 (~71us)
#   bfloat16 — 1 cyc/row, half DMA bytes, rel err ~3e-3
MM_DT = mybir.dt.bfloat16

last_exec_time_ns = None      # set when BASS_KERNEL_PROFILE=1


def _install_ntff_hook():
    """Register the axon NTFF profile hook if the image's antenv lacks it."""
    try:
        from antenv.axon_hooks import get_axon_ntff_profile_hook  # noqa: F401
        return
    except ImportError:
        pass
    try:
        from trn_agent_boot.trn_boot import _ntff_profile_via_ctypes
        hook = _ntff_profile_via_ctypes("/opt/axon/libaxon_pjrt.so")
    except Exception:
        hook = None
    mod = types.ModuleType("antenv.axon_hooks")
    mod.get_axon_ntff_profile_hook = lambda: hook
    mod.set_axon_ntff_profile_hook = lambda h: None
    sys.modules["antenv.axon_hooks"] = mod


def _np_mm_dtype():
    if MM_DT == mybir.dt.bfloat16:
        import ml_dtypes
        return np.dtype(ml_dtypes.bfloat16)
    return np.dtype(np.float32)


def fold_conv_into_fc(conv_w: np.ndarray, w0: np.ndarray) -> np.ndarray:
    """W_eff[200,784] such that x @ W_eff.T == fc1(flatten(conv(x)))."""
    w0v = w0.reshape(HID, 26, 26).astype(np.float64)
    w_img = np.zeros((HID, 28, 28), dtype=np.float64)
    for ki in range(3):
        for kj in range(3):
            w_img[:, ki:ki + 26, kj:kj + 26] += w0v * np.float64(conv_w[ki, kj])
    return w_img.reshape(HID, KDIM).astype(np.float32)


def pack_shard(xs: np.ndarray, mm_np):
    """Pack one x shard [4096, 784] into per-group SBUF tile images.

    Group g (gsz chunks starting at chunk c0):
      xg[p, j, a, n] = x[(c0+j)*CHUNK + n, a*KT + p]
    Every SBUF partition line is one contiguous (j, a, n) run.
    """
    xsv = xs.reshape(SHARD, NKT, KT)
    arrays = []
    c0 = 0
    for w in SEGS:
        blk = xsv[c0:c0 + w]                        # [n, a, p]
        arrays.append(np.ascontiguousarray(
            blk.transpose(2, 1, 0).astype(mm_np)))  # [p, a, n]
        c0 += w
    return arrays


def pack_weights(w_eff: np.ndarray, w1: np.ndarray, b0, b1, mm_np):
    """Pack weights/biases into single-DMA SBUF images."""
    # w0sb[p, a, m] = W_eff[m, a*KT + p]
    w0sb = np.ascontiguousarray(
        w_eff.reshape(HID, NKT, KT).transpose(2, 1, 0).astype(mm_np))
    # w1sb[p, 0:10] = w1[:, p].T ; w1sb[0:72, 10:20] = w1[:, 128+p].T
    w1sb = np.zeros((128, 2 * OUT), dtype=mm_np)
    w1sb[:, :OUT] = w1[:, 0:128].T.astype(mm_np)
    w1sb[:HID - 128, OUT:] = w1[:, 128:HID].T.astype(mm_np)
    # bias[p, 0] = b0[p]; bias[0:72, 1] = b0[128:200]; bias[0:10, 2] = b1
    biases = np.zeros((128, 3), dtype=np.float32)
    biases[:, 0] = b0[0:128]
    biases[:HID - 128, 1] = b0[128:HID]
    biases[:OUT, 2] = b1
    return w0sb, w1sb, biases


def build_program():
    nc = bacc.Bacc("TRN2", target_bir_lowering=False, debug=False)
    f32 = mybir.dt.float32
    add = mybir.AluOpType.add
    amax = mybir.AluOpType.max

    xg_d = [
        nc.declare_dram_parameter(
            f"xg{g}", [KT, NKT, w], MM_DT, isOutput=False)
        for g, w in enumerate(SEGS)
    ]
    w0_d = nc.declare_dram_parameter("w0sb", [KT, NKT, HID], MM_DT, isOutput=False)
    w1_d = nc.declare_dram_parameter("w1sb", [128, 2 * OUT], MM_DT, isOutput=False)
    bia_d = nc.declare_dram_parameter("biases", [128, 3], f32, isOutput=False)
    out_d = nc.declare_dram_parameter("out", [OUT, SHARD], f32, isOutput=True)

    with tile.TileContext(nc) as tc:
        with (
            tc.tile_pool(name="weights", bufs=1) as wpool,
            tc.tile_pool(name="xin", bufs=3) as xpool,
            tc.tile_pool(name="hbuf", bufs=2) as hpool,
            tc.tile_pool(name="obuf", bufs=4) as opool,
            tc.tile_pool(name="psum", bufs=2, space=bass.MemorySpace.PSUM) as pp,
            tc.tile_pool(name="opsum", bufs=2, space=bass.MemorySpace.PSUM) as op,
        ):
            # weights + biases ride the ACT ring so the SP ring belongs
            # exclusively to the x stream (first-chunk completion time)
            w0 = wpool.tile([KT, NKT, HID], MM_DT)
            nc.scalar.dma_start(w0[:], w0_d[:])
            bia = wpool.tile([128, 3], f32)
            nc.scalar.dma_start(bia[:], bia_d[:])
            w1 = wpool.tile([128, 2 * OUT], MM_DT)
            nc.scalar.dma_start(w1[:], w1_d[:])

            # PE pre-warm on zeroed scratch while the first DMAs fly
            warm_x = wpool.tile([KT, CHUNK], MM_DT)
            nc.gpsimd.memset(warm_x[:], 0.0)
            warm_ps = op.tile([128, CHUNK], f32, tag="warm", bufs=1)
            for _ in range(N_WARMUP):
                nc.tensor.matmul(
                    warm_ps[:], warm_x[:, 0:128], warm_x[:],
                    start=True, stop=True)

            def emit_layer2(g, w, c0, h_tiles):
                # layer 2: outT[10, seg], 2 accumulating matmuls
                o_ps = op.tile([OUT, w], f32, tag="ops", name=f"ops_{g}")
                nc.tensor.matmul(
                    o_ps[:], w1[0:128, 0:OUT], h_tiles[0][:],
                    start=True, stop=False)
                nc.tensor.matmul(
                    o_ps[:], w1[0:HID - 128, OUT:2 * OUT], h_tiles[1][:],
                    start=False, stop=True)
                o_sb = opool.tile([OUT, w], f32, tag="osb", name=f"osb_{g}")
                # bias-add on the scalar engine, off DVE's queue
                nc.scalar.activation(
                    o_sb[:], o_ps[:],
                    mybir.ActivationFunctionType.Identity,
                    bias=bia[0:OUT, 2:3])
                # output store on the SWDGE ring (keeps HWDGE free); the
                # last two ride the idle SP ring — SWDGE's end-of-kernel
                # drain of a just-issued store costs ~2us
                if g >= len(SEGS) - 2:
                    nc.sync.dma_start(out_d[:, c0:c0 + w], o_sb[:])
                else:
                    nc.gpsimd.dma_start(out_d[:, c0:c0 + w], o_sb[:])

            c0 = 0
            x_dmas = []
            pending = None   # layer 2 runs one segment behind layer 1,
            # so the PE never waits on the DVE relu at a seg boundary
            for g, w in enumerate(SEGS):
                xg = xpool.tile([KT, NKT, w], MM_DT, tag="xg",
                                name=f"xg_{g}")
                # mid-stream segments ride the SWDGE ring for extra
                # aggregate bandwidth; the rest use the SP HWDGE ring
                eng = nc.gpsimd if g in (4, 6) else nc.sync
                dma = eng.dma_start(xg[:], xg_d[g][:])
                # cap x-DMA queue depth: the SDMA engines round-robin
                # across queued transfers, so deeper queues delay the
                # completion of the segment the PE needs next
                if g >= 2:
                    tile.add_dep_helper(
                        dma.ins, x_dmas[g - 2].ins, sync=True,
                        reason="throttle x DMA in-flight depth to 2")
                x_dmas.append(dma)

                # layer 1: hT[m0:m0+dm, seg], 7 accumulating matmuls
                h_tiles = []
                for mi, (m0, dm) in enumerate(M_TILES):
                    h_ps = pp.tile([dm, w], f32, tag=f"hps{mi}",
                                   name=f"hps_{g}_{mi}")
                    for a in range(NKT):
                        nc.tensor.matmul(
                            h_ps[:],
                            w0[:, a, m0:m0 + dm],
                            xg[:, a, :],
                            start=(a == 0),
                            stop=(a == NKT - 1),
                        )
                    h_sb = hpool.tile([dm, w], MM_DT, tag=f"h{mi}",
                                      name=f"h_{g}_{mi}")
                    # fused bias + relu on the vector engine
                    nc.vector.tensor_scalar(
                        h_sb[:], h_ps[:], bia[0:dm, mi:mi + 1], 0.0,
                        add, amax)
                    h_tiles.append(h_sb)

                if pending is not None:
                    emit_layer2(*pending)
                pending = (g, w, c0, h_tiles)
                c0 += w

            emit_layer2(*pending)

    nc.compile()
    return nc


_program_cache = {}


def _get_program():
    key = (MM_DT, tuple(SEGS), N_WARMUP)
    if key not in _program_cache:
        _program_cache[key] = build_program()
    return _program_cache[key]


def kernel(**inputs: np.ndarray) -> np.ndarray:
    x = np.asarray(inputs["x"], dtype=np.float32)
    conv_w = np.asarray(inputs["conv_w"], dtype=np.float32)
    w0 = np.asarray(inputs["w0"], dtype=np.float32)
    b0 = np.asarray(inputs["b0"], dtype=np.float32)
    w1 = np.asarray(inputs["w1"], dtype=np.float32)
    b1 = np.asarray(inputs["b1"], dtype=np.float32)

    mm_np = _np_mm_dtype()
    w_eff = fold_conv_into_fc(conv_w, w0)
    w0sb, w1sb, biases = pack_weights(w_eff, w1, b0, b1, mm_np)

    in_maps = []
    for i in range(N_CORES):
        xgs = pack_shard(x[i * SHARD:(i + 1) * SHARD], mm_np)
        m = {f"xg{g}": xg for g, xg in enumerate(xgs)}
        m.update({"w0sb": w0sb, "w1sb": w1sb, "biases": biases})
        in_maps.append(m)

    nc = _get_program()

    profile = os.environ.get("BASS_KERNEL_PROFILE", "0") == "1"
    kwargs = {}
    if profile:
        _install_ntff_hook()
        kwargs = dict(trace=True, tmpdir=os.environ.get("BASS_KERNEL_TRACE_DIR"))
    try:
        res = run_bass_kernel_spmd(
            nc, in_maps, core_ids=list(range(N_CORES)), **kwargs)
    except Exception:
        # a previous process can leave a NeuronCore momentarily
        # unrecoverable (NRT_EXEC_UNIT_UNRECOVERABLE); one retry suffices
        import time
        time.sleep(5)
        res = run_bass_kernel_spmd(
            nc, in_maps, core_ids=list(range(N_CORES)), **kwargs)

    global last_exec_time_ns
    last_exec_time_ns = res.exec_time_ns

    out = np.empty((B, OUT), dtype=np.float32)
    for i in range(N_CORES):
        out[i * SHARD:(i + 1) * SHARD] = res.results[i]["out"].T
    return out
